# revision 9
# baseline (speedup 1.0000x reference)
"""Multi-class 3D DICE loss on 8 Trainium2 NeuronCores.

Data-parallel over the subject (batch) axis: core b reduces subject b's
[C=4, 64, 128, 128] volumes to per-class (inter, mask_sum, out_sum) partial
sums; the host applies the ~10-flop DICE scalar tail and averages the 8
per-subject losses.

Per-core layout: each input tensor is viewed as [128, 32768] where
partition q = c*32 + p (class c in partition block [32c, 32c+32)).
Per chunk (descending sizes, 4 MiB DMAs in steady state), one engine per
reduction so no engine exceeds ~45us against the ~80us DMA stream:
  - DVE  scalar_tensor_tensor: partial sums of output*masks   (inter)
  - ACT  activation(Copy, accum_out): partial sums of masks   (msum)
  - PE   fp32r matmuls vs the class indicator, PSUM-accumulated
         across all 512-col slices: per-class column sums of x (xsum)
The final collapse: one fp32 matmul folds acc partition blocks into
per-class (inter, msum); DVE reduces the PSUM tiles to a [C, 3] output.
"""

import os
import sys
from contextlib import ExitStack

import numpy as np

for _p in ("/opt/trn_rl_repo",):
    if _p not in sys.path and os.path.isdir(_p):
        sys.path.insert(0, _p)

import concourse.bass as bass  # noqa: E402
import concourse.tile as tile  # noqa: E402
from concourse import bacc, mybir  # noqa: E402
from concourse.bass_utils import run_bass_kernel_spmd  # noqa: E402

N_CORES = 8
B, C = 8, 4
SPATIAL = 64 * 128 * 128            # 1,048,576 per (subject, class)
P = 128                             # SBUF partitions = C * 32
COLS = (C * SPATIAL) // P           # 32768 elements per partition
# Descending chunk schedule: big DMAs (4 MiB) for bandwidth in the steady
# state, small chunks at the end so the post-last-byte compute tail is tiny.
CHUNKS = [8192, 8192, 8192, 4096, 2048, 1024, 512, 512]
BIG_FD = 4096  # chunks >= this land in the big pools, the rest in tail pools
MM = 512       # fp32r matmul slice width (moving-operand max, full-rate N>=256)
assert sum(CHUNKS) == COLS
NCHUNK = len(CHUNKS)
N_SLICES = COLS // MM
EPS = 1e-7
F32 = mybir.dt.float32
F32R = mybir.dt.float32r


def _dice_body(ctx: ExitStack, tc: "tile.TileContext", out_ap, x_ap, m_ap, ind_ap):
    nc = tc.nc
    add = mybir.AluOpType.add
    mult = mybir.AluOpType.mult
    Copy = mybir.ActivationFunctionType.Copy

    consts = ctx.enter_context(tc.tile_pool(name="consts", bufs=1))
    xpool = ctx.enter_context(tc.tile_pool(name="xin", bufs=2))
    mpool = ctx.enter_context(tc.tile_pool(name="min", bufs=2))
    xtail = ctx.enter_context(tc.tile_pool(name="xtail", bufs=3))
    mtail = ctx.enter_context(tc.tile_pool(name="mtail", bufs=3))
    small = ctx.enter_context(tc.tile_pool(name="small", bufs=1))
    psum = ctx.enter_context(tc.tile_pool(name="psum", bufs=1, space="PSUM"))

    # Block indicator: ind[q, c] = 1.0 iff q // 32 == c. lhsT for the
    # partition-block -> per-class collapse (exact in any matmul precision).
    ind = consts.tile([P, C], F32)
    nc.vector.memset(ind[:], 0.0)
    for c in range(C):
        nc.vector.memset(ind[c * 32 : (c + 1) * 32, c : c + 1], 1.0)
    # fp32r copy for the slice matmuls (memset can't write f32r; DMA can).
    ind_r = consts.tile([P, C], F32R, tag="ind_r")
    nc.sync.dma_start(out=ind_r[:], in_=ind_ap[:])

    # Per-chunk partial sums (column j <- chunk j); no cross-chunk deps.
    # Cols [0,N) = sum(x*m) on DVE, [N,2N) = sum(m) on ACT — each engine
    # owns a full 32 B accumulator word (mixing engines within one word
    # produced intermittent lost-update corruption on HW).
    acc = small.tile([P, 3 * NCHUNK], F32)
    # Engines must write their full elementwise result somewhere; stride-0
    # broadcast dummies avoid real [P, fd] scratch tiles (HW-verified).
    dve_dummy = small.tile([P, 1], F32)
    act_dummy = small.tile([P, 1], F32)
    # PE accumulates per-class x column sums across every 512-col slice.
    ps_x = psum.tile([C, MM], F32)

    off = 0
    sl = 0
    for j, fd in enumerate(CHUNKS):
        big = fd >= BIG_FD
        xt = (xpool if big else xtail).tile([P, fd], F32R, tag="xt")
        nc.sync.dma_start(out=xt[:], in_=x_ap[:, off : off + fd])
        mt = (mpool if big else mtail).tile([P, fd], F32, tag="mt")
        nc.sync.dma_start(out=mt[:], in_=m_ap[:, off : off + fd])
        off += fd

        # inter partials on DVE: out = (x*1)*m, accum = X-reduce(out).
        nc.vector.scalar_tensor_tensor(
            out=dve_dummy.broadcast_to((P, fd)),
            in0=xt[:].bitcast(F32),
            scalar=1.0,
            in1=mt[:],
            op0=mult,
            op1=mult,
            accum_out=acc[:, j : j + 1],
        )
        nc.scalar.activation(
            out=act_dummy.broadcast_to((P, fd)),
            in_=mt[:],
            func=Copy,
            accum_out=acc[:, NCHUNK + j : NCHUNK + j + 1],
        )
        # x-sums: tensor_scalar with accum (engine A/B test)
        nc.vector.tensor_scalar(
            out=dve_dummy.broadcast_to((P, fd)),
            in0=xt[:].bitcast(F32),
            scalar1=1.0,
            scalar2=0.0,
            op0=mult,
            op1=add,
            accum_out=acc[:, 2 * NCHUNK + j : 2 * NCHUNK + j + 1],
        )

    # Partition blocks -> per-(class, quantity, chunk) sums in one matmul,
    # then PSUM-side reduces -> [4, 3] class sums (inter, msum, xsum). The
    # remaining ~10-flop scalar tail runs on the host during unshard.
    ps2 = psum.tile([C, 3 * NCHUNK], F32)
    nc.tensor.matmul(out=ps2[:], lhsT=ind[:], rhs=acc[:], start=True, stop=True)
    sums = small.tile([C, 3], F32)
    nc.vector.tensor_reduce(
        sums[:],
        ps2[:].rearrange("c (q n) -> c q n", q=3),
        axis=mybir.AxisListType.X,
        op=add,
    )
    nc.sync.dma_start(out=out_ap, in_=sums[:])


_CACHE: dict[str, object] = {}


def _build():
    if "nc" in _CACHE:
        return _CACHE["nc"]
    nc = bacc.Bacc("TRN2", target_bir_lowering=False, debug=False)
    x = nc.dram_tensor("x", [P, COLS], F32R, kind="ExternalInput").ap()
    m = nc.dram_tensor("m", [P, COLS], F32, kind="ExternalInput").ap()
    ind = nc.dram_tensor("ind", [P, C], F32R, kind="ExternalInput").ap()
    out = nc.dram_tensor("class_sums", [C, 3], F32, kind="ExternalOutput").ap()
    with tile.TileContext(nc) as tc:
        with ExitStack() as ctx:
            _dice_body(ctx, tc, out, x, m, ind)
    nc.compile()
    _CACHE["nc"] = nc
    return nc


_IND_NP = np.repeat(np.eye(C, dtype=np.float32), 32, axis=0)  # [128, 4]


def _in_maps(output: np.ndarray, masks: np.ndarray):
    output = np.ascontiguousarray(output, dtype=np.float32)
    masks = np.ascontiguousarray(masks, dtype=np.float32)
    return [
        {
            "x": output[b].reshape(P, COLS),
            "m": masks[b].reshape(P, COLS),
            "ind": _IND_NP,
        }
        for b in range(N_CORES)
    ]


def _finish(cs: np.ndarray) -> np.float32:
    """Per-subject scalar tail (fp32, mirrors the reference ordering).

    cs: [C, 3] device output — columns (inter, mask_sum, x_sum) per class.
    """
    cs = cs.astype(np.float32)
    inter, msum, xsum = cs[:, 0], cs[:, 1], cs[:, 2]
    w = np.float32(1.0) / (msum * msum + np.float32(EPS))
    total = xsum + msum
    nom = (w * inter).sum(dtype=np.float32)
    den = (w * total + np.float32(EPS)).sum(dtype=np.float32)
    return np.float32(1.0) - np.float32(2.0) * nom / den


def run_sharded(output: np.ndarray, masks: np.ndarray, **spmd_kwargs):
    """Run the SPMD kernel; returns (loss[1], BassKernelResults)."""
    nc = _build()
    res = run_bass_kernel_spmd(
        nc, _in_maps(output, masks), list(range(N_CORES)), **spmd_kwargs
    )
    per_subj = np.array(
        [_finish(res.results[b]["class_sums"]) for b in range(N_CORES)],
        dtype=np.float32,
    )
    loss = (per_subj.sum(dtype=np.float32) / np.float32(B)).reshape(1)
    return loss.astype(np.float32), res


def kernel(output: np.ndarray, masks: np.ndarray) -> np.ndarray:
    loss, _ = run_sharded(output, masks)
    return loss


# revision 15
# speedup vs baseline: 1.1136x; 1.1136x over previous
"""Multi-class 3D DICE loss on 8 Trainium2 NeuronCores — unequal sharding.

The chip's aggregate HBM bandwidth saturates when all 8 cores stream
concurrently, and arbitration is uneven: core 6 sustains ~320 GB/s while
the others get ~410 GB/s. Equal byte-sharding therefore leaves the graded
max-core time ~25% above the mean. Fix: shard the 128 units (one unit =
2048 columns of the per-subject [128, 32768] view; 16 units per subject)
unevenly — 17 units for the fast cores, 15 for cores 4/5, 13 for core 6 —
as three compiled variants dispatched concurrently on custom jax meshes.

Each core's shard is 1-2 contiguous SEGMENTS, each inside one subject
(fast cores: one full subject + 1 spare unit of another). Per segment the
kernel emits per-class (inter, mask_sum, x_sum); the host regroups
segments by subject and applies the ~10-flop DICE tail.

Per-chunk engine split (descending chunk sizes, 4 MiB DMAs in steady
state; no engine exceeds ~45us against the ~85us DMA stream):
  - DVE  scalar_tensor_tensor: partial sums of output*masks   (inter)
  - ACT  activation(Copy, accum_out): partial sums of masks   (msum)
  - PE   fp32r matmuls vs the class indicator, PSUM-accumulated per
         segment: per-class column sums of x                  (xsum)
"""

import math
import os
import sys
from contextlib import ExitStack

import numpy as np

for _p in ("/opt/trn_rl_repo",):
    if _p not in sys.path and os.path.isdir(_p):
        sys.path.insert(0, _p)

import concourse.bass as bass  # noqa: E402
import concourse.tile as tile  # noqa: E402
from concourse import bacc, bass2jax, mybir  # noqa: E402

N_CORES = 8
B, C = 8, 4
SPATIAL = 64 * 128 * 128            # 1,048,576 per (subject, class)
P = 128                             # SBUF partitions = C * 32
SUBJ_COLS = (C * SPATIAL) // P      # 32768 columns per subject
UNIT = 2048                         # shard granularity (1 MiB per tensor)
SUBJ_UNITS = SUBJ_COLS // UNIT      # 16
MM = 512                            # fp32r matmul slice (full-rate N>=256)
EPS = 1e-7
F32 = mybir.dt.float32
F32R = mybir.dt.float32r

# Variant name -> (chunk schedule, chunks-per-segment). Chunk boundaries
# are aligned so no chunk straddles a segment boundary; descending sizes
# keep steady-state DMAs big and the post-last-byte compute tail small.
VARIANTS = {
    "v17": dict(
        chunks=[8192, 8192, 8192, 4096, 2048, 1024, 512, 512, 1024, 512, 512],
        seg_nchunks=(8, 3),
    ),
    "v15": dict(
        chunks=[8192, 8192, 8192, 2048, 2048, 1024, 512, 512],
        seg_nchunks=(8,),
    ),
    "v13": dict(
        chunks=[8192, 8192, 4096, 2048, 2048, 1024, 512, 512],
        seg_nchunks=(8,),
    ),
}
for _v in VARIANTS.values():
    assert sum(_v["chunks"]) % UNIT == 0
    assert sum(_v["seg_nchunks"]) == len(_v["chunks"])

# core -> (variant, [(subject, unit_start, n_units), ...]) — contiguous
# unit ranges, each within one subject; together they tile all 128 units.
ASSIGN = {
    0: ("v17", [(0, 0, 16), (5, 15, 1)]),
    1: ("v17", [(1, 0, 16), (6, 15, 1)]),
    2: ("v17", [(2, 0, 16), (7, 13, 1)]),
    3: ("v17", [(3, 0, 16), (7, 14, 1)]),
    7: ("v17", [(4, 0, 16), (7, 15, 1)]),
    4: ("v15", [(5, 0, 15)]),
    5: ("v15", [(6, 0, 15)]),
    6: ("v13", [(7, 0, 13)]),
}
GROUPS = {  # variant -> device ids, dispatch order: largest first
    "v17": [0, 1, 2, 3, 7],
    "v15": [4, 5],
    "v13": [6],
}


def _check_assign():
    cover = np.zeros((B, SUBJ_UNITS), dtype=int)
    for core, (vname, segs) in ASSIGN.items():
        v = VARIANTS[vname]
        starts = [sum(v["seg_nchunks"][:i]) for i in range(len(v["seg_nchunks"]))]
        seg_cols = [
            sum(v["chunks"][a : a + ns])
            for a, ns in zip(starts, v["seg_nchunks"])
        ]
        assert len(segs) == len(seg_cols)
        for (sub, us, n), cols in zip(segs, seg_cols):
            assert n * UNIT == cols, (core, vname, n * UNIT, cols)
            cover[sub, us : us + n] += 1
    assert (cover == 1).all()


_check_assign()


def _dice_body(ctx, tc, out_ap, x_ap, m_ap, ind_ap, chunks, seg_nchunks):
    nc = tc.nc
    add = mybir.AluOpType.add
    mult = mybir.AluOpType.mult
    Copy = mybir.ActivationFunctionType.Copy
    NCH = len(chunks)
    NSEG = len(seg_nchunks)
    PADN = 8 * math.ceil(NCH / 8)  # keep each engine's accum cols in own 32B words
    seg_start = [sum(seg_nchunks[:i]) for i in range(NSEG)]  # first chunk of seg
    seg_of = []
    for s, ns in enumerate(seg_nchunks):
        seg_of += [s] * ns
    seg_slices = [
        sum(chunks[seg_start[s] : seg_start[s] + ns]) // MM
        for s, ns in enumerate(seg_nchunks)
    ]

    consts = ctx.enter_context(tc.tile_pool(name="consts", bufs=1))
    xpool = ctx.enter_context(tc.tile_pool(name="xin", bufs=2))
    mpool = ctx.enter_context(tc.tile_pool(name="min", bufs=2))
    xtail = ctx.enter_context(tc.tile_pool(name="xtail", bufs=3))
    mtail = ctx.enter_context(tc.tile_pool(name="mtail", bufs=3))
    small = ctx.enter_context(tc.tile_pool(name="small", bufs=1))
    psum = ctx.enter_context(tc.tile_pool(name="psum", bufs=1, space="PSUM"))

    # Block indicator: ind[q, c] = 1.0 iff q // 32 == c. lhsT for the
    # partition-block -> per-class collapse (exact in any matmul precision).
    ind = consts.tile([P, C], F32)
    nc.vector.memset(ind[:], 0.0)
    for c in range(C):
        nc.vector.memset(ind[c * 32 : (c + 1) * 32, c : c + 1], 1.0)
    # fp32r copy for the slice matmuls (memset can't write f32r; DMA can).
    ind_r = consts.tile([P, C], F32R, tag="ind_r")
    nc.sync.dma_start(out=ind_r[:], in_=ind_ap[:])

    # Per-chunk partial sums (column j <- chunk j); no cross-chunk deps.
    # Cols [0,PADN) = sum(x*m) on DVE, [PADN,2*PADN) = sum(m) on ACT — each
    # engine owns full 32 B accumulator words (mixing engines within one
    # word produced intermittent lost-update corruption on HW). Zero the
    # pad columns so the collapse matmul never reads uninitialized SBUF.
    acc = small.tile([P, 2 * PADN], F32)
    nc.vector.memset(acc[:], 0.0)
    # Engines must write their full elementwise result somewhere; stride-0
    # broadcast dummies avoid real [P, fd] scratch tiles (HW-verified).
    dve_dummy = small.tile([P, 1], F32)
    act_dummy = small.tile([P, 1], F32)
    # PE accumulates per-class x column sums across each segment's slices.
    ps_x = []
    for s in range(NSEG):
        ps_x_s = psum.tile([C, MM], F32, tag=f"ps_x{s}")
        ps_x.append(ps_x_s)

    off = 0
    sl_in_seg = 0
    for j, fd in enumerate(chunks):
        seg = seg_of[j]
        if j > 0 and seg_of[j - 1] != seg:
            sl_in_seg = 0
        big = fd >= 4096
        xt = (xpool if big else xtail).tile([P, fd], F32R, tag="xt")
        nc.sync.dma_start(out=xt[:], in_=x_ap[:, off : off + fd])
        mt = (mpool if big else mtail).tile([P, fd], F32, tag="mt")
        nc.sync.dma_start(out=mt[:], in_=m_ap[:, off : off + fd])
        off += fd

        # inter partials on DVE: out = (x*1)*m, accum = X-reduce(out).
        nc.vector.scalar_tensor_tensor(
            out=dve_dummy.broadcast_to((P, fd)),
            in0=xt[:].bitcast(F32),
            scalar=1.0,
            in1=mt[:],
            op0=mult,
            op1=mult,
            accum_out=acc[:, j : j + 1],
        )
        nc.scalar.activation(
            out=act_dummy.broadcast_to((P, fd)),
            in_=mt[:],
            func=Copy,
            accum_out=acc[:, PADN + j : PADN + j + 1],
        )
        # x-sums on PE: ps_x[seg][c, i] += sum_q ind[q, c] * x[q, s*MM+i],
        # accumulated in PSUM across the segment's slices. fp32r runs the
        # 512-wide moving operand at full rate.
        for s in range(fd // MM):
            nc.tensor.matmul(
                out=ps_x[seg][:],
                lhsT=ind_r[:],
                rhs=xt[:, s * MM : (s + 1) * MM],
                start=(sl_in_seg == 0),
                stop=(sl_in_seg == seg_slices[seg] - 1),
            )
            sl_in_seg += 1

    # Partition blocks -> per-(class, quantity, chunk) sums in one matmul,
    # then per-segment PSUM-side reduces -> [C, 3*NSEG] segment sums
    # (inter, msum, xsum per segment). The remaining ~10-flop scalar tail
    # runs on the host during unshard.
    ps2 = psum.tile([C, 2 * PADN], F32)
    nc.tensor.matmul(out=ps2[:], lhsT=ind[:], rhs=acc[:], start=True, stop=True)
    sums = small.tile([C, 3 * NSEG], F32)
    for s, ns in enumerate(seg_nchunks):
        a = seg_start[s]
        nc.vector.tensor_reduce(
            sums[:, 3 * s : 3 * s + 1],
            ps2[:, a : a + ns],
            axis=mybir.AxisListType.X,
            op=add,
        )
        nc.vector.tensor_reduce(
            sums[:, 3 * s + 1 : 3 * s + 2],
            ps2[:, PADN + a : PADN + a + ns],
            axis=mybir.AxisListType.X,
            op=add,
        )
        nc.vector.tensor_reduce(
            sums[:, 3 * s + 2 : 3 * s + 3],
            ps_x[s][:],
            axis=mybir.AxisListType.X,
            op=add,
        )
    nc.sync.dma_start(out=out_ap, in_=sums[:])


_CACHE: dict[str, object] = {}


def _build(vname: str):
    key = f"nc_{vname}"
    if key in _CACHE:
        return _CACHE[key]
    v = VARIANTS[vname]
    cols = sum(v["chunks"])
    nseg = len(v["seg_nchunks"])
    nc = bacc.Bacc("TRN2", target_bir_lowering=False, debug=False)
    x = nc.dram_tensor("x", [P, cols], F32R, kind="ExternalInput").ap()
    m = nc.dram_tensor("m", [P, cols], F32, kind="ExternalInput").ap()
    ind = nc.dram_tensor("ind", [P, C], F32R, kind="ExternalInput").ap()
    out = nc.dram_tensor("seg_sums", [C, 3 * nseg], F32, kind="ExternalOutput").ap()
    with tile.TileContext(nc) as tc:
        with ExitStack() as ctx:
            _dice_body(ctx, tc, out, x, m, ind, v["chunks"], v["seg_nchunks"])
    nc.compile()
    _CACHE[key] = nc
    return nc


def _runner(vname: str):
    """Jitted shard_map runner for a variant on its assigned devices."""
    key = f"run_{vname}"
    if key in _CACHE:
        return _CACHE[key]
    import jax
    from jax.experimental.shard_map import shard_map
    from jax.sharding import Mesh, PartitionSpec

    bass2jax.install_neuronx_cc_hook()
    nc = _build(vname)
    device_ids = GROUPS[vname]

    partition_name = (
        nc.partition_id_tensor.name if nc.partition_id_tensor else None
    )
    in_names, out_names, out_avals, zero_outs = [], [], [], []
    for alloc in nc.m.functions[0].allocations:
        if not isinstance(alloc, mybir.MemoryLocationSet):
            continue
        name = alloc.memorylocations[0].name
        if alloc.kind == "ExternalInput":
            if name != partition_name:
                in_names.append(name)
        elif alloc.kind == "ExternalOutput":
            out_names.append(name)
            shape = tuple(alloc.tensor_shape)
            dtype = mybir.dt.np(alloc.dtype)
            out_avals.append(jax.core.ShapedArray(shape, dtype))
            zero_outs.append(np.zeros(shape, dtype))
    n_params = len(in_names)
    n_outs = len(out_avals)
    all_in_names = in_names + out_names
    if partition_name is not None:
        all_in_names.append(partition_name)
    donate = tuple(range(n_params, n_params + n_outs))

    def _body(*args):
        operands = list(args)
        if partition_name is not None:
            operands.append(bass2jax.partition_id_tensor())
        outs = bass2jax._bass_exec_p.bind(
            *operands,
            out_avals=tuple(out_avals),
            in_names=tuple(all_in_names),
            out_names=tuple(out_names),
            lowering_input_output_aliases=(),
            sim_require_finite=True,
            sim_require_nnan=True,
            nc=nc,
        )
        return tuple(outs)

    devices = [jax.devices()[i] for i in device_ids]
    n = len(devices)
    mesh = Mesh(np.asarray(devices), ("core",))
    in_specs = (PartitionSpec("core"),) * (n_params + n_outs)
    out_specs = (PartitionSpec("core"),) * n_outs
    sharded = jax.jit(
        shard_map(_body, mesh=mesh, in_specs=in_specs, out_specs=out_specs,
                  check_rep=False),
        donate_argnums=donate,
        keep_unused=True,
    )

    def run(in_maps):
        assert len(in_maps) == n
        per_core = [[np.asarray(m_[nm]) for nm in in_names] for m_ in in_maps]
        concat_in = [
            np.concatenate([per_core[c][i] for c in range(n)], axis=0)
            for i in range(n_params)
        ]
        concat_zeros = [
            np.zeros((n * z.shape[0], *z.shape[1:]), z.dtype) for z in zero_outs
        ]
        out_arrs = sharded(*concat_in, *concat_zeros)

        def gather():
            return [
                {
                    name: np.asarray(out_arrs[i]).reshape(n, *out_avals[i].shape)[c]
                    for i, name in enumerate(out_names)
                }
                for c in range(n)
            ]

        return gather

    _CACHE[key] = run
    return run


_IND_NP = np.repeat(np.eye(C, dtype=np.float32), 32, axis=0)  # [128, 4]


def _core_inputs(output: np.ndarray, masks: np.ndarray, core: int):
    _, segs = ASSIGN[core]
    xs, ms = [], []
    for sub, us, n in segs:
        lo, hi = us * UNIT, (us + n) * UNIT
        xs.append(output[sub].reshape(P, SUBJ_COLS)[:, lo:hi])
        ms.append(masks[sub].reshape(P, SUBJ_COLS)[:, lo:hi])
    return {
        "x": np.ascontiguousarray(np.concatenate(xs, axis=1)),
        "m": np.ascontiguousarray(np.concatenate(ms, axis=1)),
        "ind": _IND_NP,
    }


def run_split(output: np.ndarray, masks: np.ndarray):
    """Dispatch all three variants concurrently; returns (loss[1], groups)
    where groups = [(vname, nc, device_ids)] for the profiler."""
    output = np.ascontiguousarray(output, dtype=np.float32)
    masks = np.ascontiguousarray(masks, dtype=np.float32)

    gathers = []
    for vname, ids in GROUPS.items():  # v17 first (largest shards)
        run = _runner(vname)
        gathers.append(
            (vname, ids, run([_core_inputs(output, masks, c) for c in ids]))
        )

    # [B, C, 3] per-subject class sums assembled from segment partials.
    subj = np.zeros((B, C, 3), dtype=np.float32)
    for vname, ids, gather in gathers:
        results = gather()
        for slot, core in enumerate(ids):
            _, segs = ASSIGN[core]
            seg_sums = results[slot]["seg_sums"].astype(np.float32)  # [C, 3*NSEG]
            for s, (sub, _, _) in enumerate(segs):
                subj[sub] += seg_sums[:, 3 * s : 3 * s + 3]

    per_subj = np.array([_finish(subj[b]) for b in range(B)], dtype=np.float32)
    loss = (per_subj.sum(dtype=np.float32) / np.float32(B)).reshape(1)
    groups = [(vname, _CACHE[f"nc_{vname}"], ids) for vname, ids in GROUPS.items()]
    return loss.astype(np.float32), groups


def _finish(cs: np.ndarray) -> np.float32:
    """Per-subject scalar tail (fp32, mirrors the reference ordering).

    cs: [C, 3] — columns (inter, mask_sum, x_sum) per class.
    """
    cs = cs.astype(np.float32)
    inter, msum, xsum = cs[:, 0], cs[:, 1], cs[:, 2]
    w = np.float32(1.0) / (msum * msum + np.float32(EPS))
    total = xsum + msum
    nom = (w * inter).sum(dtype=np.float32)
    den = (w * total + np.float32(EPS)).sum(dtype=np.float32)
    return np.float32(1.0) - np.float32(2.0) * nom / den


def kernel(output: np.ndarray, masks: np.ndarray) -> np.ndarray:
    loss, _ = run_split(output, masks)
    return loss


# revision 16
# speedup vs baseline: 1.1327x; 1.0172x over previous
"""Multi-class 3D DICE loss on 8 Trainium2 NeuronCores — unequal sharding.

The chip's aggregate HBM bandwidth saturates when all 8 cores stream
concurrently, and arbitration is uneven: core 6 sustains ~320 GB/s while
the others get ~410 GB/s. Equal byte-sharding therefore leaves the graded
max-core time ~25% above the mean. Fix: shard the 128 units (one unit =
2048 columns of the per-subject [128, 32768] view; 16 units per subject)
unevenly — 17 units for the fast cores, 15 for cores 4/5, 13 for core 6 —
as three compiled variants dispatched concurrently on custom jax meshes.

Each core's shard is 1-2 contiguous SEGMENTS, each inside one subject
(fast cores: one full subject + 1 spare unit of another). Per segment the
kernel emits per-class (inter, mask_sum, x_sum); the host regroups
segments by subject and applies the ~10-flop DICE tail.

Per-chunk engine split (descending chunk sizes, 4 MiB DMAs in steady
state; no engine exceeds ~45us against the ~85us DMA stream):
  - DVE  scalar_tensor_tensor: partial sums of output*masks   (inter)
  - ACT  activation(Copy, accum_out): partial sums of masks   (msum)
  - PE   fp32r matmuls vs the class indicator, PSUM-accumulated per
         segment: per-class column sums of x                  (xsum)
"""

import math
import os
import sys
from contextlib import ExitStack

import numpy as np

for _p in ("/opt/trn_rl_repo",):
    if _p not in sys.path and os.path.isdir(_p):
        sys.path.insert(0, _p)

import concourse.bass as bass  # noqa: E402
import concourse.tile as tile  # noqa: E402
from concourse import bacc, bass2jax, mybir  # noqa: E402

N_CORES = 8
B, C = 8, 4
SPATIAL = 64 * 128 * 128            # 1,048,576 per (subject, class)
P = 128                             # SBUF partitions = C * 32
SUBJ_COLS = (C * SPATIAL) // P      # 32768 columns per subject
UNIT = 2048                         # shard granularity (1 MiB per tensor)
SUBJ_UNITS = SUBJ_COLS // UNIT      # 16
MM = 512                            # fp32r matmul slice (full-rate N>=256)
EPS = 1e-7
F32 = mybir.dt.float32
F32R = mybir.dt.float32r

# Variant name -> (chunk schedule, chunks-per-segment). Chunk boundaries
# are aligned so no chunk straddles a segment boundary; descending sizes
# keep steady-state DMAs big and the post-last-byte compute tail small.
VARIANTS = {
    "v16": dict(
        chunks=[8192, 8192, 8192, 4096, 2048, 2048],
        seg_nchunks=(6,),
    ),
}

# core -> (variant, [(subject, unit_start, n_units), ...]) — contiguous
# unit ranges, each within one subject; together they tile all 128 units.
ASSIGN = {c: ("v16", [(c, 0, 16)]) for c in range(8)}
GROUPS = {  # variant -> device ids
    "v16": [0, 1, 2, 3, 4, 5, 6, 7],
}


def _check_assign():
    cover = np.zeros((B, SUBJ_UNITS), dtype=int)
    for core, (vname, segs) in ASSIGN.items():
        v = VARIANTS[vname]
        starts = [sum(v["seg_nchunks"][:i]) for i in range(len(v["seg_nchunks"]))]
        seg_cols = [
            sum(v["chunks"][a : a + ns])
            for a, ns in zip(starts, v["seg_nchunks"])
        ]
        assert len(segs) == len(seg_cols)
        for (sub, us, n), cols in zip(segs, seg_cols):
            assert n * UNIT == cols, (core, vname, n * UNIT, cols)
            cover[sub, us : us + n] += 1
    assert (cover == 1).all()


_check_assign()


def _dice_body(ctx, tc, out_ap, x_ap, m_ap, ind_ap, chunks, seg_nchunks):
    nc = tc.nc
    add = mybir.AluOpType.add
    mult = mybir.AluOpType.mult
    Copy = mybir.ActivationFunctionType.Copy
    NCH = len(chunks)
    NSEG = len(seg_nchunks)
    PADN = 8 * math.ceil(NCH / 8)  # keep each engine's accum cols in own 32B words
    seg_start = [sum(seg_nchunks[:i]) for i in range(NSEG)]  # first chunk of seg
    seg_of = []
    for s, ns in enumerate(seg_nchunks):
        seg_of += [s] * ns
    seg_slices = [
        sum(chunks[seg_start[s] : seg_start[s] + ns]) // MM
        for s, ns in enumerate(seg_nchunks)
    ]

    consts = ctx.enter_context(tc.tile_pool(name="consts", bufs=1))
    xpool = ctx.enter_context(tc.tile_pool(name="xin", bufs=2))
    mpool = ctx.enter_context(tc.tile_pool(name="min", bufs=2))
    xtail = ctx.enter_context(tc.tile_pool(name="xtail", bufs=3))
    mtail = ctx.enter_context(tc.tile_pool(name="mtail", bufs=3))
    small = ctx.enter_context(tc.tile_pool(name="small", bufs=1))
    psum = ctx.enter_context(tc.tile_pool(name="psum", bufs=1, space="PSUM"))

    # Block indicator: ind[q, c] = 1.0 iff q // 32 == c. lhsT for the
    # partition-block -> per-class collapse (exact in any matmul precision).
    ind = consts.tile([P, C], F32)
    nc.vector.memset(ind[:], 0.0)
    for c in range(C):
        nc.vector.memset(ind[c * 32 : (c + 1) * 32, c : c + 1], 1.0)
    # fp32r copy for the slice matmuls (memset can't write f32r; DMA can).
    ind_r = consts.tile([P, C], F32R, tag="ind_r")
    nc.sync.dma_start(out=ind_r[:], in_=ind_ap[:])

    # Per-chunk partial sums (column j <- chunk j); no cross-chunk deps.
    # Cols [0,PADN) = sum(x*m) on DVE, [PADN,2*PADN) = sum(m) on ACT — each
    # engine owns full 32 B accumulator words (mixing engines within one
    # word produced intermittent lost-update corruption on HW). Zero the
    # pad columns so the collapse matmul never reads uninitialized SBUF.
    acc = small.tile([P, 2 * PADN], F32)
    nc.vector.memset(acc[:], 0.0)
    # Engines must write their full elementwise result somewhere; stride-0
    # broadcast dummies avoid real [P, fd] scratch tiles (HW-verified).
    dve_dummy = small.tile([P, 1], F32)
    act_dummy = small.tile([P, 1], F32)
    # PE accumulates per-class x column sums across each segment's slices.
    ps_x = []
    for s in range(NSEG):
        ps_x_s = psum.tile([C, MM], F32, tag=f"ps_x{s}")
        ps_x.append(ps_x_s)

    off = 0
    sl_in_seg = 0
    for j, fd in enumerate(chunks):
        seg = seg_of[j]
        if j > 0 and seg_of[j - 1] != seg:
            sl_in_seg = 0
        big = fd >= 4096
        xt = (xpool if big else xtail).tile([P, fd], F32R, tag="xt")
        nc.sync.dma_start(out=xt[:], in_=x_ap[:, off : off + fd])
        mt = (mpool if big else mtail).tile([P, fd], F32, tag="mt")
        nc.sync.dma_start(out=mt[:], in_=m_ap[:, off : off + fd])
        off += fd

        # inter partials on DVE: out = (x*1)*m, accum = X-reduce(out).
        nc.vector.scalar_tensor_tensor(
            out=dve_dummy.broadcast_to((P, fd)),
            in0=xt[:].bitcast(F32),
            scalar=1.0,
            in1=mt[:],
            op0=mult,
            op1=mult,
            accum_out=acc[:, j : j + 1],
        )
        nc.scalar.activation(
            out=act_dummy.broadcast_to((P, fd)),
            in_=mt[:],
            func=Copy,
            accum_out=acc[:, PADN + j : PADN + j + 1],
        )
        # x-sums on PE: ps_x[seg][c, i] += sum_q ind[q, c] * x[q, s*MM+i],
        # accumulated in PSUM across the segment's slices. fp32r runs the
        # 512-wide moving operand at full rate.
        for s in range(fd // MM):
            nc.tensor.matmul(
                out=ps_x[seg][:],
                lhsT=ind_r[:],
                rhs=xt[:, s * MM : (s + 1) * MM],
                start=(sl_in_seg == 0),
                stop=(sl_in_seg == seg_slices[seg] - 1),
            )
            sl_in_seg += 1

    # Partition blocks -> per-(class, quantity, chunk) sums in one matmul,
    # then per-segment PSUM-side reduces -> [C, 3*NSEG] segment sums
    # (inter, msum, xsum per segment). The remaining ~10-flop scalar tail
    # runs on the host during unshard.
    ps2 = psum.tile([C, 2 * PADN], F32)
    nc.tensor.matmul(out=ps2[:], lhsT=ind[:], rhs=acc[:], start=True, stop=True)
    sums = small.tile([C, 3 * NSEG], F32)
    for s, ns in enumerate(seg_nchunks):
        a = seg_start[s]
        nc.vector.tensor_reduce(
            sums[:, 3 * s : 3 * s + 1],
            ps2[:, a : a + ns],
            axis=mybir.AxisListType.X,
            op=add,
        )
        nc.vector.tensor_reduce(
            sums[:, 3 * s + 1 : 3 * s + 2],
            ps2[:, PADN + a : PADN + a + ns],
            axis=mybir.AxisListType.X,
            op=add,
        )
        nc.vector.tensor_reduce(
            sums[:, 3 * s + 2 : 3 * s + 3],
            ps_x[s][:],
            axis=mybir.AxisListType.X,
            op=add,
        )
    nc.sync.dma_start(out=out_ap, in_=sums[:])


_CACHE: dict[str, object] = {}


def _build(vname: str):
    key = f"nc_{vname}"
    if key in _CACHE:
        return _CACHE[key]
    v = VARIANTS[vname]
    cols = sum(v["chunks"])
    nseg = len(v["seg_nchunks"])
    nc = bacc.Bacc("TRN2", target_bir_lowering=False, debug=False)
    x = nc.dram_tensor("x", [P, cols], F32R, kind="ExternalInput").ap()
    m = nc.dram_tensor("m", [P, cols], F32, kind="ExternalInput").ap()
    ind = nc.dram_tensor("ind", [P, C], F32R, kind="ExternalInput").ap()
    out = nc.dram_tensor("seg_sums", [C, 3 * nseg], F32, kind="ExternalOutput").ap()
    with tile.TileContext(nc) as tc:
        with ExitStack() as ctx:
            _dice_body(ctx, tc, out, x, m, ind, v["chunks"], v["seg_nchunks"])
    nc.compile()
    _CACHE[key] = nc
    return nc


def _runner(vname: str):
    """Jitted shard_map runner for a variant on its assigned devices."""
    key = f"run_{vname}"
    if key in _CACHE:
        return _CACHE[key]
    import jax
    from jax.experimental.shard_map import shard_map
    from jax.sharding import Mesh, PartitionSpec

    bass2jax.install_neuronx_cc_hook()
    nc = _build(vname)
    device_ids = GROUPS[vname]

    partition_name = (
        nc.partition_id_tensor.name if nc.partition_id_tensor else None
    )
    in_names, out_names, out_avals, zero_outs = [], [], [], []
    for alloc in nc.m.functions[0].allocations:
        if not isinstance(alloc, mybir.MemoryLocationSet):
            continue
        name = alloc.memorylocations[0].name
        if alloc.kind == "ExternalInput":
            if name != partition_name:
                in_names.append(name)
        elif alloc.kind == "ExternalOutput":
            out_names.append(name)
            shape = tuple(alloc.tensor_shape)
            dtype = mybir.dt.np(alloc.dtype)
            out_avals.append(jax.core.ShapedArray(shape, dtype))
            zero_outs.append(np.zeros(shape, dtype))
    n_params = len(in_names)
    n_outs = len(out_avals)
    all_in_names = in_names + out_names
    if partition_name is not None:
        all_in_names.append(partition_name)
    donate = tuple(range(n_params, n_params + n_outs))

    def _body(*args):
        operands = list(args)
        if partition_name is not None:
            operands.append(bass2jax.partition_id_tensor())
        outs = bass2jax._bass_exec_p.bind(
            *operands,
            out_avals=tuple(out_avals),
            in_names=tuple(all_in_names),
            out_names=tuple(out_names),
            lowering_input_output_aliases=(),
            sim_require_finite=True,
            sim_require_nnan=True,
            nc=nc,
        )
        return tuple(outs)

    devices = [jax.devices()[i] for i in device_ids]
    n = len(devices)
    mesh = Mesh(np.asarray(devices), ("core",))
    in_specs = (PartitionSpec("core"),) * (n_params + n_outs)
    out_specs = (PartitionSpec("core"),) * n_outs
    sharded = jax.jit(
        shard_map(_body, mesh=mesh, in_specs=in_specs, out_specs=out_specs,
                  check_rep=False),
        donate_argnums=donate,
        keep_unused=True,
    )

    def run(in_maps):
        assert len(in_maps) == n
        per_core = [[np.asarray(m_[nm]) for nm in in_names] for m_ in in_maps]
        concat_in = [
            np.concatenate([per_core[c][i] for c in range(n)], axis=0)
            for i in range(n_params)
        ]
        concat_zeros = [
            np.zeros((n * z.shape[0], *z.shape[1:]), z.dtype) for z in zero_outs
        ]
        out_arrs = sharded(*concat_in, *concat_zeros)

        def gather():
            return [
                {
                    name: np.asarray(out_arrs[i]).reshape(n, *out_avals[i].shape)[c]
                    for i, name in enumerate(out_names)
                }
                for c in range(n)
            ]

        return gather

    _CACHE[key] = run
    return run


_IND_NP = np.repeat(np.eye(C, dtype=np.float32), 32, axis=0)  # [128, 4]


def _core_inputs(output: np.ndarray, masks: np.ndarray, core: int):
    _, segs = ASSIGN[core]
    xs, ms = [], []
    for sub, us, n in segs:
        lo, hi = us * UNIT, (us + n) * UNIT
        xs.append(output[sub].reshape(P, SUBJ_COLS)[:, lo:hi])
        ms.append(masks[sub].reshape(P, SUBJ_COLS)[:, lo:hi])
    return {
        "x": np.ascontiguousarray(np.concatenate(xs, axis=1)),
        "m": np.ascontiguousarray(np.concatenate(ms, axis=1)),
        "ind": _IND_NP,
    }


def run_split(output: np.ndarray, masks: np.ndarray):
    """Dispatch all three variants concurrently; returns (loss[1], groups)
    where groups = [(vname, nc, device_ids)] for the profiler."""
    output = np.ascontiguousarray(output, dtype=np.float32)
    masks = np.ascontiguousarray(masks, dtype=np.float32)

    gathers = []
    for vname, ids in GROUPS.items():  # v17 first (largest shards)
        run = _runner(vname)
        gathers.append(
            (vname, ids, run([_core_inputs(output, masks, c) for c in ids]))
        )

    # [B, C, 3] per-subject class sums assembled from segment partials.
    subj = np.zeros((B, C, 3), dtype=np.float32)
    for vname, ids, gather in gathers:
        results = gather()
        for slot, core in enumerate(ids):
            _, segs = ASSIGN[core]
            seg_sums = results[slot]["seg_sums"].astype(np.float32)  # [C, 3*NSEG]
            for s, (sub, _, _) in enumerate(segs):
                subj[sub] += seg_sums[:, 3 * s : 3 * s + 3]

    per_subj = np.array([_finish(subj[b]) for b in range(B)], dtype=np.float32)
    loss = (per_subj.sum(dtype=np.float32) / np.float32(B)).reshape(1)
    groups = [(vname, _CACHE[f"nc_{vname}"], ids) for vname, ids in GROUPS.items()]
    return loss.astype(np.float32), groups


def _finish(cs: np.ndarray) -> np.float32:
    """Per-subject scalar tail (fp32, mirrors the reference ordering).

    cs: [C, 3] — columns (inter, mask_sum, x_sum) per class.
    """
    cs = cs.astype(np.float32)
    inter, msum, xsum = cs[:, 0], cs[:, 1], cs[:, 2]
    w = np.float32(1.0) / (msum * msum + np.float32(EPS))
    total = xsum + msum
    nom = (w * inter).sum(dtype=np.float32)
    den = (w * total + np.float32(EPS)).sum(dtype=np.float32)
    return np.float32(1.0) - np.float32(2.0) * nom / den


def kernel(output: np.ndarray, masks: np.ndarray) -> np.ndarray:
    loss, _ = run_split(output, masks)
    return loss


# revision 17
# speedup vs baseline: 1.2941x; 1.1425x over previous
"""Multi-class 3D DICE loss on 8 Trainium2 NeuronCores — unequal sharding.

The chip's aggregate HBM bandwidth saturates when all 8 cores stream
concurrently, and arbitration is uneven: core 6 sustains ~320 GB/s while
the others get ~410 GB/s. Equal byte-sharding therefore leaves the graded
max-core time ~25% above the mean. Fix: shard the 128 units (one unit =
2048 columns of the per-subject [128, 32768] view; 16 units per subject)
unevenly — 17 units for the fast cores, 15 for cores 4/5, 13 for core 6 —
as three compiled variants dispatched concurrently on custom jax meshes.

Each core's shard is 1-2 contiguous SEGMENTS, each inside one subject
(fast cores: one full subject + 1 spare unit of another). Per segment the
kernel emits per-class (inter, mask_sum, x_sum); the host regroups
segments by subject and applies the ~10-flop DICE tail.

Per-chunk engine split (descending chunk sizes, 4 MiB DMAs in steady
state; no engine exceeds ~45us against the ~85us DMA stream):
  - DVE  scalar_tensor_tensor: partial sums of output*masks   (inter)
  - ACT  activation(Copy, accum_out): partial sums of masks   (msum)
  - PE   fp32r matmuls vs the class indicator, PSUM-accumulated per
         segment: per-class column sums of x                  (xsum)
"""

import math
import os
import sys
from contextlib import ExitStack

import numpy as np

for _p in ("/opt/trn_rl_repo",):
    if _p not in sys.path and os.path.isdir(_p):
        sys.path.insert(0, _p)

import concourse.bass as bass  # noqa: E402
import concourse.tile as tile  # noqa: E402
from concourse import bacc, bass2jax, mybir  # noqa: E402

N_CORES = 8
B, C = 8, 4
SPATIAL = 64 * 128 * 128            # 1,048,576 per (subject, class)
P = 128                             # SBUF partitions = C * 32
SUBJ_COLS = (C * SPATIAL) // P      # 32768 columns per subject
UNIT = 2048                         # shard granularity (1 MiB per tensor)
SUBJ_UNITS = SUBJ_COLS // UNIT      # 16
MM = 512                            # fp32r matmul slice (full-rate N>=256)
EPS = 1e-7
F32 = mybir.dt.float32
F32R = mybir.dt.float32r

# Variant name -> (chunk schedule, chunks-per-segment). Chunk boundaries
# are aligned so no chunk straddles a segment boundary; descending sizes
# keep steady-state DMAs big and the post-last-byte compute tail small.
VARIANTS = {
    "v17o": dict(
        chunks=[8192, 8192, 8192, 4096, 2048, 2048, 2048],
        seg_nchunks=(6, 1),
    ),
    "v15e": dict(
        chunks=[8192, 8192, 8192, 4096, 2048],
        seg_nchunks=(5,),
    ),
}

# core -> (variant, [(subject, unit_start, n_units), ...]) — contiguous
# unit ranges, each within one subject; together they tile all 128 units.
# Even cores get 15 units, odd cores 17: the sporadic late-stream stall
# only ever hits EVEN cores (observed across every profiled run) and the
# magnitude matches interference that begins once a sibling core
# finishes; evens finishing ~10us before odds keeps them clear of it.
ASSIGN = {
    0: ("v15e", [(0, 0, 15)]),
    2: ("v15e", [(2, 0, 15)]),
    4: ("v15e", [(4, 0, 15)]),
    6: ("v15e", [(6, 0, 15)]),
    1: ("v17o", [(1, 0, 16), (0, 15, 1)]),
    3: ("v17o", [(3, 0, 16), (2, 15, 1)]),
    5: ("v17o", [(5, 0, 16), (4, 15, 1)]),
    7: ("v17o", [(7, 0, 16), (6, 15, 1)]),
}
GROUPS = {  # variant -> device ids, dispatch order: largest first
    "v17o": [1, 3, 5, 7],
    "v15e": [0, 2, 4, 6],
}


def _check_assign():
    cover = np.zeros((B, SUBJ_UNITS), dtype=int)
    for core, (vname, segs) in ASSIGN.items():
        v = VARIANTS[vname]
        starts = [sum(v["seg_nchunks"][:i]) for i in range(len(v["seg_nchunks"]))]
        seg_cols = [
            sum(v["chunks"][a : a + ns])
            for a, ns in zip(starts, v["seg_nchunks"])
        ]
        assert len(segs) == len(seg_cols)
        for (sub, us, n), cols in zip(segs, seg_cols):
            assert n * UNIT == cols, (core, vname, n * UNIT, cols)
            cover[sub, us : us + n] += 1
    assert (cover == 1).all()


_check_assign()


def _dice_body(ctx, tc, out_ap, x_ap, m_ap, ind_ap, chunks, seg_nchunks):
    nc = tc.nc
    add = mybir.AluOpType.add
    mult = mybir.AluOpType.mult
    Copy = mybir.ActivationFunctionType.Copy
    NCH = len(chunks)
    NSEG = len(seg_nchunks)
    PADN = 8 * math.ceil(NCH / 8)  # keep each engine's accum cols in own 32B words
    seg_start = [sum(seg_nchunks[:i]) for i in range(NSEG)]  # first chunk of seg
    seg_of = []
    for s, ns in enumerate(seg_nchunks):
        seg_of += [s] * ns
    seg_slices = [
        sum(chunks[seg_start[s] : seg_start[s] + ns]) // MM
        for s, ns in enumerate(seg_nchunks)
    ]

    consts = ctx.enter_context(tc.tile_pool(name="consts", bufs=1))
    xpool = ctx.enter_context(tc.tile_pool(name="xin", bufs=2))
    mpool = ctx.enter_context(tc.tile_pool(name="min", bufs=2))
    xtail = ctx.enter_context(tc.tile_pool(name="xtail", bufs=3))
    mtail = ctx.enter_context(tc.tile_pool(name="mtail", bufs=3))
    small = ctx.enter_context(tc.tile_pool(name="small", bufs=1))
    psum = ctx.enter_context(tc.tile_pool(name="psum", bufs=1, space="PSUM"))

    # Block indicator: ind[q, c] = 1.0 iff q // 32 == c. lhsT for the
    # partition-block -> per-class collapse (exact in any matmul precision).
    ind = consts.tile([P, C], F32)
    nc.vector.memset(ind[:], 0.0)
    for c in range(C):
        nc.vector.memset(ind[c * 32 : (c + 1) * 32, c : c + 1], 1.0)
    # fp32r copy for the slice matmuls (memset can't write f32r; DMA can).
    ind_r = consts.tile([P, C], F32R, tag="ind_r")
    nc.sync.dma_start(out=ind_r[:], in_=ind_ap[:])

    # Per-chunk partial sums (column j <- chunk j); no cross-chunk deps.
    # Cols [0,PADN) = sum(x*m) on DVE, [PADN,2*PADN) = sum(m) on ACT — each
    # engine owns full 32 B accumulator words (mixing engines within one
    # word produced intermittent lost-update corruption on HW). Zero the
    # pad columns so the collapse matmul never reads uninitialized SBUF.
    acc = small.tile([P, 2 * PADN], F32)
    nc.vector.memset(acc[:], 0.0)
    # Engines must write their full elementwise result somewhere; stride-0
    # broadcast dummies avoid real [P, fd] scratch tiles (HW-verified).
    dve_dummy = small.tile([P, 1], F32)
    act_dummy = small.tile([P, 1], F32)
    # PE accumulates per-class x column sums across each segment's slices.
    ps_x = []
    for s in range(NSEG):
        ps_x_s = psum.tile([C, MM], F32, tag=f"ps_x{s}")
        ps_x.append(ps_x_s)

    off = 0
    sl_in_seg = 0
    for j, fd in enumerate(chunks):
        seg = seg_of[j]
        if j > 0 and seg_of[j - 1] != seg:
            sl_in_seg = 0
        big = fd >= 4096
        xt = (xpool if big else xtail).tile([P, fd], F32R, tag="xt")
        nc.sync.dma_start(out=xt[:], in_=x_ap[:, off : off + fd])
        mt = (mpool if big else mtail).tile([P, fd], F32, tag="mt")
        nc.sync.dma_start(out=mt[:], in_=m_ap[:, off : off + fd])
        off += fd

        # inter partials on DVE: out = (x*1)*m, accum = X-reduce(out).
        nc.vector.scalar_tensor_tensor(
            out=dve_dummy.broadcast_to((P, fd)),
            in0=xt[:].bitcast(F32),
            scalar=1.0,
            in1=mt[:],
            op0=mult,
            op1=mult,
            accum_out=acc[:, j : j + 1],
        )
        nc.scalar.activation(
            out=act_dummy.broadcast_to((P, fd)),
            in_=mt[:],
            func=Copy,
            accum_out=acc[:, PADN + j : PADN + j + 1],
        )
        # x-sums on PE: ps_x[seg][c, i] += sum_q ind[q, c] * x[q, s*MM+i],
        # accumulated in PSUM across the segment's slices. fp32r runs the
        # 512-wide moving operand at full rate.
        for s in range(fd // MM):
            nc.tensor.matmul(
                out=ps_x[seg][:],
                lhsT=ind_r[:],
                rhs=xt[:, s * MM : (s + 1) * MM],
                start=(sl_in_seg == 0),
                stop=(sl_in_seg == seg_slices[seg] - 1),
            )
            sl_in_seg += 1

    # Partition blocks -> per-(class, quantity, chunk) sums in one matmul,
    # then per-segment PSUM-side reduces -> [C, 3*NSEG] segment sums
    # (inter, msum, xsum per segment). The remaining ~10-flop scalar tail
    # runs on the host during unshard.
    ps2 = psum.tile([C, 2 * PADN], F32)
    nc.tensor.matmul(out=ps2[:], lhsT=ind[:], rhs=acc[:], start=True, stop=True)
    sums = small.tile([C, 3 * NSEG], F32)
    for s, ns in enumerate(seg_nchunks):
        a = seg_start[s]
        nc.vector.tensor_reduce(
            sums[:, 3 * s : 3 * s + 1],
            ps2[:, a : a + ns],
            axis=mybir.AxisListType.X,
            op=add,
        )
        nc.vector.tensor_reduce(
            sums[:, 3 * s + 1 : 3 * s + 2],
            ps2[:, PADN + a : PADN + a + ns],
            axis=mybir.AxisListType.X,
            op=add,
        )
        nc.vector.tensor_reduce(
            sums[:, 3 * s + 2 : 3 * s + 3],
            ps_x[s][:],
            axis=mybir.AxisListType.X,
            op=add,
        )
    nc.sync.dma_start(out=out_ap, in_=sums[:])


_CACHE: dict[str, object] = {}


def _build(vname: str):
    key = f"nc_{vname}"
    if key in _CACHE:
        return _CACHE[key]
    v = VARIANTS[vname]
    cols = sum(v["chunks"])
    nseg = len(v["seg_nchunks"])
    nc = bacc.Bacc("TRN2", target_bir_lowering=False, debug=False)
    x = nc.dram_tensor("x", [P, cols], F32R, kind="ExternalInput").ap()
    m = nc.dram_tensor("m", [P, cols], F32, kind="ExternalInput").ap()
    ind = nc.dram_tensor("ind", [P, C], F32R, kind="ExternalInput").ap()
    out = nc.dram_tensor("seg_sums", [C, 3 * nseg], F32, kind="ExternalOutput").ap()
    with tile.TileContext(nc) as tc:
        with ExitStack() as ctx:
            _dice_body(ctx, tc, out, x, m, ind, v["chunks"], v["seg_nchunks"])
    nc.compile()
    _CACHE[key] = nc
    return nc


def _runner(vname: str):
    """Jitted shard_map runner for a variant on its assigned devices."""
    key = f"run_{vname}"
    if key in _CACHE:
        return _CACHE[key]
    import jax
    from jax.experimental.shard_map import shard_map
    from jax.sharding import Mesh, PartitionSpec

    bass2jax.install_neuronx_cc_hook()
    nc = _build(vname)
    device_ids = GROUPS[vname]

    partition_name = (
        nc.partition_id_tensor.name if nc.partition_id_tensor else None
    )
    in_names, out_names, out_avals, zero_outs = [], [], [], []
    for alloc in nc.m.functions[0].allocations:
        if not isinstance(alloc, mybir.MemoryLocationSet):
            continue
        name = alloc.memorylocations[0].name
        if alloc.kind == "ExternalInput":
            if name != partition_name:
                in_names.append(name)
        elif alloc.kind == "ExternalOutput":
            out_names.append(name)
            shape = tuple(alloc.tensor_shape)
            dtype = mybir.dt.np(alloc.dtype)
            out_avals.append(jax.core.ShapedArray(shape, dtype))
            zero_outs.append(np.zeros(shape, dtype))
    n_params = len(in_names)
    n_outs = len(out_avals)
    all_in_names = in_names + out_names
    if partition_name is not None:
        all_in_names.append(partition_name)
    donate = tuple(range(n_params, n_params + n_outs))

    def _body(*args):
        operands = list(args)
        if partition_name is not None:
            operands.append(bass2jax.partition_id_tensor())
        outs = bass2jax._bass_exec_p.bind(
            *operands,
            out_avals=tuple(out_avals),
            in_names=tuple(all_in_names),
            out_names=tuple(out_names),
            lowering_input_output_aliases=(),
            sim_require_finite=True,
            sim_require_nnan=True,
            nc=nc,
        )
        return tuple(outs)

    devices = [jax.devices()[i] for i in device_ids]
    n = len(devices)
    mesh = Mesh(np.asarray(devices), ("core",))
    in_specs = (PartitionSpec("core"),) * (n_params + n_outs)
    out_specs = (PartitionSpec("core"),) * n_outs
    sharded = jax.jit(
        shard_map(_body, mesh=mesh, in_specs=in_specs, out_specs=out_specs,
                  check_rep=False),
        donate_argnums=donate,
        keep_unused=True,
    )

    def run(in_maps):
        assert len(in_maps) == n
        per_core = [[np.asarray(m_[nm]) for nm in in_names] for m_ in in_maps]
        concat_in = [
            np.concatenate([per_core[c][i] for c in range(n)], axis=0)
            for i in range(n_params)
        ]
        concat_zeros = [
            np.zeros((n * z.shape[0], *z.shape[1:]), z.dtype) for z in zero_outs
        ]
        out_arrs = sharded(*concat_in, *concat_zeros)

        def gather():
            return [
                {
                    name: np.asarray(out_arrs[i]).reshape(n, *out_avals[i].shape)[c]
                    for i, name in enumerate(out_names)
                }
                for c in range(n)
            ]

        return gather

    _CACHE[key] = run
    return run


_IND_NP = np.repeat(np.eye(C, dtype=np.float32), 32, axis=0)  # [128, 4]


def _core_inputs(output: np.ndarray, masks: np.ndarray, core: int):
    _, segs = ASSIGN[core]
    xs, ms = [], []
    for sub, us, n in segs:
        lo, hi = us * UNIT, (us + n) * UNIT
        xs.append(output[sub].reshape(P, SUBJ_COLS)[:, lo:hi])
        ms.append(masks[sub].reshape(P, SUBJ_COLS)[:, lo:hi])
    return {
        "x": np.ascontiguousarray(np.concatenate(xs, axis=1)),
        "m": np.ascontiguousarray(np.concatenate(ms, axis=1)),
        "ind": _IND_NP,
    }


def run_split(output: np.ndarray, masks: np.ndarray):
    """Dispatch all three variants concurrently; returns (loss[1], groups)
    where groups = [(vname, nc, device_ids)] for the profiler."""
    output = np.ascontiguousarray(output, dtype=np.float32)
    masks = np.ascontiguousarray(masks, dtype=np.float32)

    gathers = []
    for vname, ids in GROUPS.items():  # v17 first (largest shards)
        run = _runner(vname)
        gathers.append(
            (vname, ids, run([_core_inputs(output, masks, c) for c in ids]))
        )

    # [B, C, 3] per-subject class sums assembled from segment partials.
    subj = np.zeros((B, C, 3), dtype=np.float32)
    for vname, ids, gather in gathers:
        results = gather()
        for slot, core in enumerate(ids):
            _, segs = ASSIGN[core]
            seg_sums = results[slot]["seg_sums"].astype(np.float32)  # [C, 3*NSEG]
            for s, (sub, _, _) in enumerate(segs):
                subj[sub] += seg_sums[:, 3 * s : 3 * s + 3]

    per_subj = np.array([_finish(subj[b]) for b in range(B)], dtype=np.float32)
    loss = (per_subj.sum(dtype=np.float32) / np.float32(B)).reshape(1)
    groups = [(vname, _CACHE[f"nc_{vname}"], ids) for vname, ids in GROUPS.items()]
    return loss.astype(np.float32), groups


def _finish(cs: np.ndarray) -> np.float32:
    """Per-subject scalar tail (fp32, mirrors the reference ordering).

    cs: [C, 3] — columns (inter, mask_sum, x_sum) per class.
    """
    cs = cs.astype(np.float32)
    inter, msum, xsum = cs[:, 0], cs[:, 1], cs[:, 2]
    w = np.float32(1.0) / (msum * msum + np.float32(EPS))
    total = xsum + msum
    nom = (w * inter).sum(dtype=np.float32)
    den = (w * total + np.float32(EPS)).sum(dtype=np.float32)
    return np.float32(1.0) - np.float32(2.0) * nom / den


def kernel(output: np.ndarray, masks: np.ndarray) -> np.ndarray:
    loss, _ = run_split(output, masks)
    return loss
